# revision 9
# baseline (speedup 1.0000x reference)
"""Trainium2 Bass kernel for nn_Deepset GNN message-passing problem.

Computation:
    h  = relu(x @ W1 + b1)          # [N, 64]   (x: [400000, 1024])
    h2 = h @ W2 + b2                # [N, 64]
    pooled = segment_mean(h2, batch, 512)
    z = (pooled @ W3 + b3) @ W4 + b4
    out = softmax(z, axis=0)        # [512, 2]

Device does the dominant work: h = relu(x@W1+b1) and a per-graph
segment-sum of h. Everything downstream of the [512, 64] segment sums
(~2 MFLOP) runs on host.

Sharding: data-parallel over nodes, 50000 nodes/core on 8 cores.
`batch` is sorted, so each core's graph ids span < 128 consecutive
values; ids are shifted into a per-core [0, 128) window on host and the
per-core partial sums S_i [128, 64] are overlap-added on host.

Device pipeline per core (fp8 compute, fp32 accumulation):
  - x shard cast to fp8 e4m3 and packed tile-major on host so each
    512-node tile is one fully contiguous [128 partitions x 4KB] DMA
  - main matmul in fp8 DoubleRow perf mode: each instr contracts 256
    features (2 fp8 weights/PE cell), so a 512-node tile needs only 4
    matmuls of N=512 instead of 8 -> ~2x tensor-engine throughput
  - tiles are processed in PAIRS: the even tile accumulates into PSUM
    partitions 0-63, the odd tile into 64-127 (tile_position col-group),
    so ScalarE relu (+bias, fp8 e3m4 out) lands both halves in one
    [128, 512] SBUF tile and ONE PE-transpose per 128-node chunk serves
    both tiles
  - segment-sum: one-hot(batch) built on DVE (iota==graph_id compare,
    fp8 out), accumulated via matmul into one PSUM bank kept open across
    the whole kernel
"""

import numpy as np

N_NODES = 400000
D_FEAT = 1024
HIDDEN = 64
NUM_GRAPHS = 512
N_CORES = 8
NPC = N_NODES // N_CORES        # 50000 nodes per core
TILE_N = 512                    # nodes per PE tile
N_PAD = 50176                   # 98 * 512
N_TILES = N_PAD // TILE_N       # 98
CHUNK = 128                     # nodes per segment-sum chunk
KC = D_FEAT // 128              # 8 contraction chunks
GWIN = 128                      # per-core graph-id window

LAST_RESULT = None              # BassKernelResults of the last run (for profiling)


def _build_nc(d_feat=D_FEAT, n_pad=N_PAD, tile_n=TILE_N, hidden=HIDDEN,
              chunk=CHUNK, gwin=GWIN, repeat=1, xp_bufs=12, dma_split=2,
              mode="full"):
    # mode: bench-only component isolation. "full" (the real kernel),
    # "notail" (no segment-sum tail), "nodma" (one hoisted x tile reused),
    # "mainonly" (notail+nodma), "dmaonly" (no compute).
    do_dma = mode in ("full", "notail", "dmaonly")
    do_main = mode != "dmaonly"
    do_tail = mode in ("full", "nodma")
    import concourse.bass as bass
    import concourse.bacc as bacc
    import concourse.tile as tile
    from concourse import mybir
    from contextlib import ExitStack

    dt = mybir.dt
    DR = mybir.MatmulPerfMode.DoubleRow
    kc = d_feat // 128
    n_tiles = n_pad // tile_n
    n_chunks = n_pad // chunk
    cpt = tile_n // chunk       # chunks per tile (4)
    n_pairs = n_tiles // 2
    assert n_tiles % 2 == 0

    nc = bacc.Bacc("TRN2", target_bir_lowering=False, debug=False)
    xT = nc.declare_dram_parameter("xT", [n_tiles, 128, kc * tile_n],
                                   dt.float8e4, isOutput=False)
    w1 = nc.declare_dram_parameter("w1", [d_feat, hidden], dt.float8e4,
                                   isOutput=False)
    b1 = nc.declare_dram_parameter("b1", [128, 1], dt.float32, isOutput=False)
    bsh = nc.declare_dram_parameter("bsh", [chunk, n_chunks], dt.float32,
                                    isOutput=False)
    iota = nc.declare_dram_parameter("iota", [chunk, gwin], dt.bfloat16,
                                     isOutput=False)
    ident = nc.declare_dram_parameter("ident", [128, 128], dt.float8e3,
                                      isOutput=False)
    sout = nc.declare_dram_parameter("sout", [gwin, hidden], dt.float32,
                                     isOutput=True)

    w1_r = w1[:, :].rearrange("(c p) h -> p c h", p=128)

    with ExitStack() as ctx:
        tc = ctx.enter_context(tile.TileContext(nc))
        const = ctx.enter_context(tc.tile_pool(name="const", bufs=1))
        xp = ctx.enter_context(tc.tile_pool(name="xp", bufs=xp_bufs))
        htp = ctx.enter_context(tc.tile_pool(name="htp", bufs=4, space=bass.MemorySpace.PSUM))
        hts = ctx.enter_context(tc.tile_pool(name="hts", bufs=3))
        hnp = ctx.enter_context(tc.tile_pool(name="hnp", bufs=2, space=bass.MemorySpace.PSUM))
        hns = ctx.enter_context(tc.tile_pool(name="hns", bufs=6))
        ohp = ctx.enter_context(tc.tile_pool(name="ohp", bufs=8))
        ssp = ctx.enter_context(tc.tile_pool(name="ssp", bufs=1, space=bass.MemorySpace.PSUM))

        w1_sb = const.tile([128, kc, hidden], dt.float8e4)
        nc.sync.dma_start(w1_sb[:], w1_r)
        b1_sb = const.tile([128, 1], dt.float32)
        nc.sync.dma_start(b1_sb[:], b1[:, :])
        bsh_sb = const.tile([chunk, n_chunks], dt.float32)
        nc.sync.dma_start(bsh_sb[:], bsh[:, :])
        iota_sb = const.tile([chunk, gwin], dt.bfloat16)
        nc.sync.dma_start(iota_sb[:], iota[:, :])
        ident_sb = const.tile([128, 128], dt.float8e3)
        nc.sync.dma_start(ident_sb[:], ident[:, :])

        # Per-graph sums accumulate in a single PSUM bank across all chunks
        # of the kernel (start only on the very first chunk).
        s_ps = ssp.tile([gwin, hidden], dt.float32)

        # Segment-sum tail for one tile pair: one PE-transpose per 128-node
        # chunk covers both tiles (even tile's h in cols 0-63, odd in
        # 64-127), one-hot built on DVE, accumulate via matmul.
        def seg_tail(r, p, hp_sb):
            for c in range(cpt):
                # fp8 transpose-mode writes on 2-byte granularity: walrus
                # requires an output element step of 2.
                h_ps = hnp.tile([128, 128, 2], dt.float8e3)
                nc.tensor.transpose(h_ps[:, :, 0], hp_sb[:, c * chunk:(c + 1) * chunk],
                                    ident_sb[:])
                h_sb = hns.tile([128, 128], dt.float8e3)
                nc.vector.tensor_copy(h_sb[:], h_ps[:, :, 0])
                for tt in range(2):
                    gc = (2 * p + tt) * cpt + c
                    oh = ohp.tile([chunk, gwin], dt.float8e3)
                    nc.vector.tensor_single_scalar(oh[:], iota_sb[:],
                                                   bsh_sb[:, gc:gc + 1],
                                                   mybir.AluOpType.is_equal)
                    nc.tensor.matmul(s_ps[:], oh[:], h_sb[:, tt * hidden:(tt + 1) * hidden],
                                     start=(r == 0 and gc == 0 and c == 0 and tt == 0 and p == 0),
                                     stop=(r == repeat - 1 and p == n_pairs - 1
                                           and c == cpt - 1 and tt == 1),
                                     skip_group_check=True)

        # PE is in-order, so pair p's transposes (gated on the ScalarE relu)
        # are emitted after pair p+1's main matmuls — the relu latency hides
        # under them instead of stalling the PE stream.
        x_fix = None
        if not do_dma and do_main:
            x_fix = xp.tile([128, kc, tile_n], dt.float8e4)
            nc.sync.dma_start(x_fix[:], xT[0, :, :].rearrange("p (c n) -> p c n", c=kc))

        pending = None
        for r in range(repeat):  # repeat>1 is a bench-only mode
            for p in range(n_pairs):
                hp_sb = None
                if do_main:
                    hp_sb = hts.tile([128, tile_n], dt.float8e3, name="hp_sb")
                for tt in range(2):
                    t = 2 * p + tt
                    if do_dma:
                        xt = xp.tile([128, kc, tile_n], dt.float8e4)
                        xsrc = xT[t, :, :].rearrange("p (c n) -> p c n", c=kc)
                        ks = kc // dma_split
                        for s in range(dma_split):
                            nc.sync.dma_start(xt[:, s * ks:(s + 1) * ks, :],
                                              xsrc[:, s * ks:(s + 1) * ks, :])
                    else:
                        xt = x_fix
                    if not do_main:
                        continue

                    # DoubleRow matmuls must write PSUM partition base 0, so
                    # each tile gets its own [64, 512] bank; ScalarE's
                    # partition-shifted write stacks the odd tile's relu into
                    # sbuf partitions 64-127 for the shared PE-transpose.
                    ht_ps = htp.tile([hidden, tile_n], dt.float32)
                    for j in range(kc // 2):
                        nc.tensor.matmul(ht_ps[:], w1_sb[:, 2 * j:2 * j + 2, :],
                                         xt[:, 2 * j:2 * j + 2, :],
                                         start=(j == 0), stop=(j == kc // 2 - 1),
                                         perf_mode=DR)

                    nc.scalar.activation(hp_sb[tt * hidden:(tt + 1) * hidden, :],
                                         ht_ps[:],
                                         mybir.ActivationFunctionType.Relu,
                                         bias=b1_sb[0:hidden, :])

                if do_tail:
                    if pending is not None:
                        seg_tail(*pending)
                    pending = (r, p, hp_sb)
        if do_tail:
            seg_tail(*pending)

        s_sb = const.tile([gwin, hidden], dt.float32)
        if do_tail:
            nc.vector.tensor_copy(s_sb[:], s_ps[:])
        else:
            nc.vector.memset(s_sb[:], 0.0)
        nc.sync.dma_start(sout[:, :], s_sb[:])

    nc.compile()
    return nc


def _prep_inputs(x, batch):
    """Per-core input maps + per-core graph-window bases."""
    import ml_dtypes
    f8 = np.dtype(ml_dtypes.float8_e4m3)
    f8t = np.dtype(ml_dtypes.float8_e3m4)
    bf16 = np.dtype(ml_dtypes.bfloat16)

    iota_np = np.ascontiguousarray(
        np.broadcast_to(np.arange(GWIN, dtype=np.float32), (CHUNK, GWIN))).astype(bf16)
    ident_np = np.eye(128, dtype=np.float32).astype(f8t)

    in_maps = []
    g_bases = []
    n_chunks = N_PAD // CHUNK
    for i in range(N_CORES):
        lo, hi = i * NPC, (i + 1) * NPC
        xs = np.zeros((N_PAD, D_FEAT), dtype=f8)
        xs[:NPC] = x[lo:hi].astype(f8)
        # tile-major pack: xTt[t, p, c*TILE_N + n] = x[t*TILE_N + n, c*128 + p]
        # so each 512-node tile is one fully-contiguous [128, 4KB] DMA.
        xT = np.ascontiguousarray(
            xs.reshape(N_TILES, TILE_N, KC, 128).transpose(0, 3, 2, 1)
        ).reshape(N_TILES, 128, KC * TILE_N)

        b = np.asarray(batch[lo:hi], dtype=np.int64)
        g0 = int(b[0])
        span = int(b[-1]) - g0
        assert span < GWIN, f"core {i}: graph span {span} >= {GWIN}"
        g_bases.append(g0)
        bshift = np.full((N_PAD,), -1.0, np.float32)
        bshift[:NPC] = (b - g0).astype(np.float32)
        bsh_np = np.ascontiguousarray(bshift.reshape(n_chunks, CHUNK).T)

        in_maps.append({
            "xT": xT,
            "bsh": bsh_np,
            "iota": iota_np,
            "ident": ident_np,
        })
    return in_maps, g_bases


def kernel(x, batch, W1, b1, W2, b2, W3, b3, W4, b4):
    global LAST_RESULT
    import ml_dtypes
    from concourse.bass_utils import run_bass_kernel_spmd

    f8 = np.dtype(ml_dtypes.float8_e4m3)
    x = np.asarray(x)
    batch = np.asarray(batch)
    W1 = np.asarray(W1, np.float32)
    b1 = np.asarray(b1, np.float32)

    in_maps, g_bases = _prep_inputs(x, batch)
    w1_np = W1.astype(f8)
    b1_np = np.concatenate([b1, b1]).reshape(128, 1).copy()
    for m in in_maps:
        m["w1"] = w1_np
        m["b1"] = b1_np

    nc = _build_nc()
    res = run_bass_kernel_spmd(nc, in_maps, list(range(N_CORES)))
    LAST_RESULT = res

    # Host-side: overlap-add per-core partial segment sums, then the tiny head.
    S = np.zeros((NUM_GRAPHS + GWIN, HIDDEN), np.float64)
    for i in range(N_CORES):
        g0 = g_bases[i]
        S[g0:g0 + GWIN] += np.asarray(res.results[i]["sout"], np.float64)
    S = S[:NUM_GRAPHS]

    cnt = np.bincount(batch.astype(np.int64), minlength=NUM_GRAPHS).astype(np.float64)
    meanh = S / np.maximum(cnt, 1.0)[:, None]
    pooled = meanh @ np.asarray(W2, np.float64) + np.asarray(b2, np.float64)
    pooled *= (cnt > 0)[:, None]  # empty graphs pool to exactly zero in the reference
    z = pooled @ np.asarray(W3, np.float64) + np.asarray(b3, np.float64)
    z = z @ np.asarray(W4, np.float64) + np.asarray(b4, np.float64)
    z -= z.max(axis=0, keepdims=True)
    e = np.exp(z)
    out = e / e.sum(axis=0, keepdims=True)
    return out.astype(np.float32)


# revision 19
# speedup vs baseline: 14.6097x; 14.6097x over previous
"""Trainium2 Bass kernel for nn_Deepset GNN message-passing problem.

Computation:
    h  = relu(x @ W1 + b1)          # [N, 64]   (x: [400000, 1024])
    h2 = h @ W2 + b2                # [N, 64]
    pooled = segment_mean(h2, batch, 512)
    z = (pooled @ W3 + b3) @ W4 + b4
    out = softmax(z, axis=0)        # [512, 2]

Device does the dominant work: h = relu(x@W1+b1) and a per-graph
segment-sum of h. Everything downstream of the [512, 64] segment sums
(~2 MFLOP) runs on host.

Sharding: data-parallel over nodes, 50000 nodes/core on 8 cores.
`batch` is sorted, so each core's graph ids span < 128 consecutive
values; ids are shifted into a per-core [0, 128) window on host and the
per-core partial sums S_i [128, 64] are overlap-added on host.

Device pipeline per core (fp8 compute, fp32 accumulation):
  - x shard cast to fp8 e4m3 and packed tile-major on host so each
    512-node tile is one fully contiguous [128 partitions x 4KB] DMA
  - main matmul in fp8 DoubleRow perf mode: each instr contracts 256
    features (2 fp8 weights/PE cell), so a 512-node tile needs only 4
    matmuls of N=512 instead of 8 -> ~2x tensor-engine throughput
  - tiles are processed in PAIRS: the even tile accumulates into PSUM
    partitions 0-63, the odd tile into 64-127 (tile_position col-group),
    so ScalarE relu (+bias, fp8 e3m4 out) lands both halves in one
    [128, 512] SBUF tile and ONE PE-transpose per 128-node chunk serves
    both tiles
  - segment-sum: one-hot(batch) built on DVE (iota==graph_id compare,
    fp8 out), accumulated via matmul into one PSUM bank kept open across
    the whole kernel
"""

import numpy as np

N_NODES = 400000
D_FEAT = 1024
HIDDEN = 64
NUM_GRAPHS = 512
N_CORES = 8
NPC = N_NODES // N_CORES        # 50000 nodes per core
TILE_N = 512                    # nodes per PE tile
N_PAD = 50176                   # 98 * 512
N_TILES = N_PAD // TILE_N       # 98
CHUNK = 128                     # nodes per segment-sum chunk
KC = D_FEAT // 128              # 8 contraction chunks
GWIN = 128                      # per-core graph-id window

LAST_RESULT = None              # BassKernelResults of the last run (for profiling)


def _build_nc(d_feat=D_FEAT, n_pad=N_PAD, tile_n=TILE_N, hidden=HIDDEN,
              chunk=CHUNK, gwin=GWIN, repeat=1, xp_bufs=12, dma_split=2,
              mode="full"):
    # mode: bench-only component isolation. "full" (the real kernel),
    # "notail" (no segment-sum tail), "nodma" (one hoisted x tile reused),
    # "mainonly" (notail+nodma), "dmaonly" (no compute), "empty" (nothing).
    do_dma = mode in ("full", "notail", "dmaonly")
    do_main = mode not in ("dmaonly", "empty")
    do_tail = mode in ("full", "nodma")
    import concourse.bass as bass
    import concourse.bacc as bacc
    import concourse.tile as tile
    from concourse import mybir
    from contextlib import ExitStack

    dt = mybir.dt
    DR = mybir.MatmulPerfMode.DoubleRow
    kc = d_feat // 128
    n_tiles = n_pad // tile_n
    n_chunks = n_pad // chunk
    cpt = tile_n // chunk       # chunks per tile (4)
    n_pairs = n_tiles // 2
    assert n_tiles % 2 == 0

    nc = bacc.Bacc("TRN2", target_bir_lowering=False, debug=False)
    xT = nc.declare_dram_parameter("xT", [n_tiles, 128, kc * tile_n],
                                   dt.float8e4, isOutput=False)
    w1 = nc.declare_dram_parameter("w1", [d_feat, hidden], dt.float8e4,
                                   isOutput=False)
    b1 = nc.declare_dram_parameter("b1", [128, 1], dt.float32, isOutput=False)
    # host-precomputed one-hot(batch) per pair: [pair, node, chunk-of-pair*gwin]
    ohT = nc.declare_dram_parameter("ohT", [n_tiles // 2, chunk, 8 * gwin],
                                    dt.float8e3, isOutput=False)
    ident = nc.declare_dram_parameter("ident", [128, 128], dt.float8e3,
                                      isOutput=False)
    sout = nc.declare_dram_parameter("sout", [gwin, hidden], dt.float32,
                                     isOutput=True)

    w1_r = w1[:, :].rearrange("(c p) h -> p c h", p=128)

    with ExitStack() as ctx:
        tc = ctx.enter_context(tile.TileContext(nc))
        const = ctx.enter_context(tc.tile_pool(name="const", bufs=1))
        xp = ctx.enter_context(tc.tile_pool(name="xp", bufs=xp_bufs))
        htp = ctx.enter_context(tc.tile_pool(name="htp", bufs=4, space=bass.MemorySpace.PSUM))
        hts = ctx.enter_context(tc.tile_pool(name="hts", bufs=3))
        hnp = ctx.enter_context(tc.tile_pool(name="hnp", bufs=3, space=bass.MemorySpace.PSUM))
        hns = ctx.enter_context(tc.tile_pool(name="hns", bufs=6))
        ohp = ctx.enter_context(tc.tile_pool(name="ohp", bufs=3))
        ssp = ctx.enter_context(tc.tile_pool(name="ssp", bufs=1, space=bass.MemorySpace.PSUM))

        w1_sb = const.tile([128, kc, hidden], dt.float8e4)
        nc.sync.dma_start(w1_sb[:], w1_r)
        b1_sb = const.tile([128, 1], dt.float32)
        nc.sync.dma_start(b1_sb[:], b1[:, :])
        ident_sb = const.tile([128, 128], dt.float8e3)
        nc.sync.dma_start(ident_sb[:], ident[:, :])

        # Per-graph sums accumulate in a single PSUM bank across all chunks
        # of the kernel (start only on the very first chunk).
        s_ps = ssp.tile([gwin, hidden], dt.float32)

        # Segment-sum tail for one tile pair: one PE-transpose per 128-node
        # chunk covers both tiles (even tile's h in cols 0-63, odd in
        # 64-127); one-hot(batch) comes precomputed from HBM; accumulate via
        # matmul into the whole-kernel PSUM bank. Transposes+copies emitted
        # first so the PE's one-hot matmuls trail the DVE copies by several
        # chunks instead of stalling on each one.
        def seg_tail(r, p, hp_sb, oh_sb):
            h_sbs = []
            for c in range(cpt):
                # fp8 transpose-mode writes on 2-byte granularity: walrus
                # requires an output element step of 2. The tile is padded to
                # a full 2KB PSUM bank so the three hnp bufs land in three
                # distinct banks (same-bank PE-write/DVE-read serializes).
                h_ps = hnp.tile([128, 8, 128, 2], dt.float8e3, name="h_ps")
                nc.tensor.transpose(h_ps[:, 0, :, 0], hp_sb[:, c * chunk:(c + 1) * chunk],
                                    ident_sb[:])
                h_sb = hns.tile([128, 128], dt.float8e3)
                nc.vector.tensor_copy(h_sb[:], h_ps[:, 0, :, 0])
                h_sbs.append(h_sb)
            for c in range(cpt):
                for tt in range(2):
                    nc.tensor.matmul(s_ps[:], oh_sb[:, tt * cpt + c, :],
                                     h_sbs[c][:, tt * hidden:(tt + 1) * hidden],
                                     start=(r == 0 and c == 0 and tt == 0 and p == 0),
                                     stop=(r == repeat - 1 and p == n_pairs - 1
                                           and c == cpt - 1 and tt == 1),
                                     skip_group_check=True)

        # PE is in-order, so pair p's transposes (gated on the ScalarE relu)
        # are emitted after pair p+1's main matmuls — the relu latency hides
        # under them instead of stalling the PE stream.
        x_fix = None
        if not do_dma and do_main:
            x_fix = xp.tile([128, kc, tile_n], dt.float8e4)
            nc.sync.dma_start(x_fix[:], xT[0, :, :].rearrange("p (c n) -> p c n", c=kc))

        pending = None
        for r in range(repeat):  # repeat>1 is a bench-only mode
            for p in range(n_pairs):
                hp_sb = None
                oh_sb = None
                if do_tail or mode == "dmaonly":
                    oh_sb = ohp.tile([chunk, 8, gwin], dt.float8e3, name="oh_sb")
                    nc.sync.dma_start(oh_sb[:], ohT[p, :, :].rearrange(
                        "n (c g) -> n c g", c=8))
                if do_main:
                    hp_sb = hts.tile([128, tile_n], dt.float8e3, name="hp_sb")
                for tt in range(2):
                    t = 2 * p + tt
                    if do_dma:
                        xt = xp.tile([128, kc, tile_n], dt.float8e4)
                        xsrc = xT[t, :, :].rearrange("p (c n) -> p c n", c=kc)
                        ks = kc // dma_split
                        for s in range(dma_split):
                            nc.sync.dma_start(xt[:, s * ks:(s + 1) * ks, :],
                                              xsrc[:, s * ks:(s + 1) * ks, :])
                    else:
                        xt = x_fix
                    if not do_main:
                        continue

                    # DoubleRow matmuls must write PSUM partition base 0, so
                    # each tile gets its own [64, 512] bank; ScalarE's
                    # partition-shifted write stacks the odd tile's relu into
                    # sbuf partitions 64-127 for the shared PE-transpose.
                    ht_ps = htp.tile([hidden, tile_n], dt.float32)
                    for j in range(kc // 2):
                        nc.tensor.matmul(ht_ps[:], w1_sb[:, 2 * j:2 * j + 2, :],
                                         xt[:, 2 * j:2 * j + 2, :],
                                         start=(j == 0), stop=(j == kc // 2 - 1),
                                         perf_mode=DR)

                    nc.scalar.activation(hp_sb[tt * hidden:(tt + 1) * hidden, :],
                                         ht_ps[:],
                                         mybir.ActivationFunctionType.Relu,
                                         bias=b1_sb[0:hidden, :])

                if do_tail:
                    if pending is not None:
                        seg_tail(*pending)
                    pending = (r, p, hp_sb, oh_sb)
        if do_tail:
            seg_tail(*pending)

        s_sb = const.tile([gwin, hidden], dt.float32)
        if do_tail:
            nc.vector.tensor_copy(s_sb[:], s_ps[:])
        else:
            nc.vector.memset(s_sb[:], 0.0)
        nc.sync.dma_start(sout[:, :], s_sb[:])

    nc.compile()
    return nc


def _prep_inputs(x, batch):
    """Per-core input maps + per-core graph-window bases."""
    import ml_dtypes
    f8 = np.dtype(ml_dtypes.float8_e4m3)
    f8t = np.dtype(ml_dtypes.float8_e3m4)

    ident_np = np.eye(128, dtype=np.float32).astype(f8t)

    in_maps = []
    g_bases = []
    n_chunks = N_PAD // CHUNK
    n_pairs = N_TILES // 2
    for i in range(N_CORES):
        lo, hi = i * NPC, (i + 1) * NPC
        xs = np.zeros((N_PAD, D_FEAT), dtype=f8)
        xs[:NPC] = x[lo:hi].astype(f8)
        # tile-major pack: xTt[t, p, c*TILE_N + n] = x[t*TILE_N + n, c*128 + p]
        # so each 512-node tile is one fully-contiguous [128, 4KB] DMA.
        xT = np.ascontiguousarray(
            xs.reshape(N_TILES, TILE_N, KC, 128).transpose(0, 3, 2, 1)
        ).reshape(N_TILES, 128, KC * TILE_N)

        b = np.asarray(batch[lo:hi], dtype=np.int64)
        g0 = int(b[0])
        span = int(b[-1]) - g0
        assert span < GWIN, f"core {i}: graph span {span} >= {GWIN}"
        g_bases.append(g0)
        bshift = np.full((N_PAD,), -1, np.int64)
        bshift[:NPC] = b - g0
        # one-hot(batch), packed per tile-pair: ohT[p, n, cc*GWIN + g] = 1 iff
        # node (8p+cc)*128 + n belongs to shifted graph g.
        oh = (bshift.reshape(n_chunks, CHUNK)[:, :, None]
              == np.arange(GWIN, dtype=np.int64)[None, None, :])
        ohT = np.ascontiguousarray(
            oh.reshape(n_pairs, 8, CHUNK, GWIN).transpose(0, 2, 1, 3)
        ).astype(f8t).reshape(n_pairs, CHUNK, 8 * GWIN)

        in_maps.append({
            "xT": xT,
            "ohT": ohT,
            "ident": ident_np,
        })
    return in_maps, g_bases


def kernel(x, batch, W1, b1, W2, b2, W3, b3, W4, b4):
    global LAST_RESULT
    import ml_dtypes
    from concourse.bass_utils import run_bass_kernel_spmd

    f8 = np.dtype(ml_dtypes.float8_e4m3)
    x = np.asarray(x)
    batch = np.asarray(batch)
    W1 = np.asarray(W1, np.float32)
    b1 = np.asarray(b1, np.float32)

    in_maps, g_bases = _prep_inputs(x, batch)
    w1_np = W1.astype(f8)
    b1_np = np.concatenate([b1, b1]).reshape(128, 1).copy()
    for m in in_maps:
        m["w1"] = w1_np
        m["b1"] = b1_np

    nc = _build_nc()
    res = run_bass_kernel_spmd(nc, in_maps, list(range(N_CORES)))
    LAST_RESULT = res

    # Host-side: overlap-add per-core partial segment sums, then the tiny head.
    S = np.zeros((NUM_GRAPHS + GWIN, HIDDEN), np.float64)
    for i in range(N_CORES):
        g0 = g_bases[i]
        S[g0:g0 + GWIN] += np.asarray(res.results[i]["sout"], np.float64)
    S = S[:NUM_GRAPHS]

    cnt = np.bincount(batch.astype(np.int64), minlength=NUM_GRAPHS).astype(np.float64)
    meanh = S / np.maximum(cnt, 1.0)[:, None]
    pooled = meanh @ np.asarray(W2, np.float64) + np.asarray(b2, np.float64)
    pooled *= (cnt > 0)[:, None]  # empty graphs pool to exactly zero in the reference
    z = pooled @ np.asarray(W3, np.float64) + np.asarray(b3, np.float64)
    z = z @ np.asarray(W4, np.float64) + np.asarray(b4, np.float64)
    z -= z.max(axis=0, keepdims=True)
    e = np.exp(z)
    out = e / e.sum(axis=0, keepdims=True)
    return out.astype(np.float32)
